# revision 80
# baseline (speedup 1.0000x reference)
"""Trainium2 Bass kernel for qk-layernorm attention (dense transformer block).

Sharding: 8 cores = 2 batches x 4 head-groups (4 heads each).  Each core
computes qkv projection (its heads only), qk-layernorm, attention, and a
partial output projection for its head slice; the host sums the 4 partials
per batch and adds b_proj.

v2 (bf16 datapath, ~355-385us vs 533-658us fp32r baseline):
 - all matmul operands bf16 (host pre-casts x/W); PSUM accumulation fp32.
   bf16 halves LDWEIGHTS time and DMA bytes; the PE HAM clock gate
   (K=4/8 cold / 8/8 warm) dominates either way, so stream-cycle count
   and PE-queue continuity are what matter.
 - v projected directly into [n, d]-per-head layout (no PE transposes)
 - single ACT table (natural_log_exp_and_others): exp for softmax,
   rsqrt(v)=exp(-0.5*ln(v+eps)) for LN, 1/den=exp(-ln(den)) for softmax
   denominators -- no DVE reciprocal (was 105us), no table thrash
   (was 49 ACT_TABLE_LOADs = 63us)
 - phase 1 job-major: each q/k projection's LN-stats chain drains while
   the next job's matmuls stream; PSUM copies split DVE/ACT
 - attention inner loop software-pipelined: S(mt) streams while exp(mt-1)
   finishes so attn@v never head-of-line-blocks the in-order PE queue;
   o_unnorm+den staged to SBUF immediately to free the accumulator banks
 - startup DMAs spread across scalar/sync/gpsimd queues (lead-in 30->15us)
 - fp8e4m3 DoubleRow attn@v was tried (−50us) but the summed per-head-
   group errors land at 2.5e-2 > the 2e-2 gate; bf16 keeps 5.5e-3.
"""

import numpy as np
import ml_dtypes

DIM = 1024
HEADS = 16
HD = 64
B = 2
N = 2048
EPS = 1e-6
N_CORES = 8
HEADS_PER_CORE = 4
PAIRS = 2          # head pairs per core
CC = 8             # contraction chunks of 128 over DIM
NT = N // 128      # 16 n/m tiles
NCH = N // 512     # 4 chunks of 512
SCALE = HD ** -0.5

BF16NP = ml_dtypes.bfloat16
PIPELINED_AV = True   # lag attn@v one m-tile behind S to hide exp latency
WARMUP_MM = 55        # dummy matmuls spanning the DMA lead-in (HAM pre-warm)
KEEPWARM = True       # dummy matmul per attn step to hold the HAM at K=8/8   # lag attn@v one m-tile behind S to hide exp latency

_prog_cache = {}


def _pin_act_table():
    """Force all ACT activations onto the natural_log_exp_and_others table.

    The greedy table-selection pass otherwise thrashes between
    exp_and_others (Exp) and natural_log (Ln) -- 49 ACT_TABLE_LOADs x
    1.28us in the traced kernel.  We keep table order/ids intact (ids are
    positional into act_info.json) but report every other table as empty
    so the one table containing {Exp, Ln, Square} serves everything.
    """
    import concourse.bacc as bacc_mod
    if getattr(bacc_mod.get_activation_tables, "_pinned", False):
        return
    orig = bacc_mod.get_activation_tables
    pref = "natural_log_exp_and_others"

    def pinned(arch):
        t = orig(arch)
        if pref not in t:
            return t
        return {name: (funcs if name == pref else set())
                for name, funcs in t.items()}

    pinned._pinned = True
    bacc_mod.get_activation_tables = pinned


def _build_program():
    import concourse.bass as bass
    import concourse.tile as tile
    from concourse import mybir, bacc

    _pin_act_table()

    F32 = mybir.dt.float32
    BF16 = mybir.dt.bfloat16
    Act = mybir.ActivationFunctionType
    Alu = mybir.AluOpType

    nc = bacc.Bacc("TRN2", target_bir_lowering=False, debug=False,
                   num_devices=N_CORES)

    # ---- DRAM I/O ----
    xT_d = nc.dram_tensor("xT", [DIM, N], BF16, kind="ExternalInput").ap()
    wqk_d = nc.dram_tensor("wqk", [DIM, 512], BF16, kind="ExternalInput").ap()
    wv_d = nc.dram_tensor("wv", [DIM, 256], BF16, kind="ExternalInput").ap()
    wp_d = nc.dram_tensor("wp", [256, DIM], BF16, kind="ExternalInput").ap()
    smu_q_d = nc.dram_tensor("smu_q", [128, 128], BF16, kind="ExternalInput").ap()
    ssq_q_d = nc.dram_tensor("ssq_q", [128, 128], BF16, kind="ExternalInput").ap()
    smu_k_d = nc.dram_tensor("smu_k", [128, 128], BF16, kind="ExternalInput").ap()
    ssq_k_d = nc.dram_tensor("ssq_k", [128, 128], BF16, kind="ExternalInput").ap()
    bsel_d = nc.dram_tensor("bsel", [128, 64], BF16, kind="ExternalInput").ap()
    gq_d = nc.dram_tensor("gq", [128, 1], F32, kind="ExternalInput").ap()
    gk_d = nc.dram_tensor("gk", [128, 1], F32, kind="ExternalInput").ap()
    ones_d = nc.dram_tensor("ones", [128, 64], BF16, kind="ExternalInput").ap()
    # y partials leave the core in bf16 (halves the 8MB output DMA); the
    # host sums the 4 head-group partials in fp32
    y_d = nc.dram_tensor("y", [N, DIM], BF16, kind="ExternalOutput").ap()

    with tile.TileContext(nc) as tc:
        with tc.tile_pool(name="wts", bufs=1) as wts, \
             tc.tile_pool(name="persist", bufs=1) as persist:
            # ---- persistent SBUF tensors ----
            # Startup DMAs are spread across engine queues so issue
            # overhead (~1us per dma_start) doesn't serialize the lead-in:
            # scalar: wqk (gates first matmuls); vector: xt chunk 0;
            # sync: wv + the rest; gpsimd: memsets.
            # zero source for HAM warm-up matmuls (memset lands ~0.5us in,
            # long before the first weight DMA completes)
            warmsrc = wts.tile([128, 512], BF16)
            nc.gpsimd.memset(warmsrc[:], 0.0)
            wqk = wts.tile([128, CC * 512], BF16)     # [c-chunk, 4 o-tiles x 128]
            wqk_r = wqk_d.rearrange("(cc p) o -> p cc o", p=128)
            ccs = slice(0, CC // 2)
            nc.scalar.dma_start(
                wqk[:].rearrange("p (cc o) -> p cc o", cc=CC)[:, ccs],
                wqk_r[:, ccs])
            wv = wts.tile([128, CC * 256], BF16)
            nc.sync.dma_start(wv[:].rearrange("p (cc o) -> p cc o", cc=CC),
                              wv_d.rearrange("(cc p) o -> p cc o", p=128))
            wp = wts.tile([128, 2 * DIM], BF16)
            smu = [wts.tile([128, 128], BF16, tag=f"smu{t}", name=f"smu{t}") for t in range(2)]
            ssq = [wts.tile([128, 128], BF16, tag=f"ssq{t}", name=f"ssq{t}") for t in range(2)]
            nc.sync.dma_start(smu[0][:], smu_q_d[:])
            nc.sync.dma_start(ssq[0][:], ssq_q_d[:])
            nc.sync.dma_start(smu[1][:], smu_k_d[:])
            nc.sync.dma_start(ssq[1][:], ssq_k_d[:])
            gcol = [wts.tile([128, 1], F32, tag=f"g{t}", name=f"g{t}") for t in range(2)]
            nc.sync.dma_start(gcol[0][:], gq_d[:])
            nc.sync.dma_start(gcol[1][:], gk_d[:])
            ones = wts.tile([128, 64], BF16)
            nc.sync.dma_start(ones[:], ones_d[:])
            bsel = wts.tile([128, 64], BF16)
            nc.sync.dma_start(bsel[:], bsel_d[:])
            # o_unnorm staging (rows 65-127 stay zero for the bsel matmul)
            ou = [wts.tile([128, 1024], BF16, tag=f"ou{h}", name=f"ou{h}")
                  for h in range(2)]
            nc.gpsimd.memset(ou[0][:], 0.0)
            nc.gpsimd.memset(ou[1][:], 0.0)
            epsb = wts.tile([128, 1], F32)
            nc.gpsimd.memset(epsb[:], EPS)

            # qk[0],qk[1]: q pair tiles; qk[2],qk[3]: k pair tiles  [d-pair, n]
            qk = [persist.tile([128, N], BF16, tag=f"qk{i}", name=f"qk{i}") for i in range(4)]
            # v in [m, per-nt: 4 heads x (64 v | 1 ones | 15 pad)] layout
            v_sb = persist.tile([128, NT * 320], BF16)
            nc.gpsimd.memset(v_sb[:], 0.0)
            onorm = [persist.tile([128, N], BF16, tag=f"on{p}", name=f"on{p}") for p in range(PAIRS)]
            v_ones = v_sb[:].rearrange("p (nt b c) -> p nt b c", b=4, c=80)[:, :, :, 64:65]
            nc.gpsimd.dma_start(
                v_ones, ones_d.rearrange("p (nt b) -> p nt b", b=4)[:, :, :, None])
            # wp is only needed by the output projection (late)
            nc.gpsimd.dma_start(wp[:].rearrange("p (pc o) -> p pc o", pc=2),
                                wp_d.rearrange("(pc p) o -> p pc o", p=128))

            # ================= Phase 1: qkv projection + qk layernorm =====
            xT_r = xT_d.rearrange("(cc p) n -> p cc n", p=128)
            # jobs: (dest qk tile, col offset in wqk)
            qk_jobs = [(qk[0], 0), (qk[2], 256), (qk[1], 128), (qk[3], 384)]

            with tc.tile_pool(name="xT", bufs=2) as xp, \
                 tc.tile_pool(name="p1tmp", bufs=2) as tmp, \
                 tc.tile_pool(name="ps_p1", bufs=1, space="PSUM") as ps:
                # The PE HAM clock gate starts at K=4/8 (1.2 GHz) and only
                # reaches 8/8 after ~3.4us of sustained activity.  Burn the
                # DMA lead-in on dummy matmuls so phase 1 starts warm.
                if WARMUP_MM:
                    warm = ps.tile([128, 512], F32, tag="acc0", name="warm")
                    for _ in range(WARMUP_MM):
                        nc.tensor.matmul(warm[0:64, :], warmsrc[:, 0:64],
                                         warmsrc[:], start=True, stop=True,
                                         skip_group_check=True)

                def ln_stats(src, sl, t):
                    # qk layernorm for one 512-token chunk of one q/k pair
                    sqc = tmp.tile([128, 512], BF16, tag="sqc", name="sqc")
                    nc.vector.tensor_mul(sqc[:], src[:, sl], src[:, sl])
                    pmu = ps.tile([128, 512], F32, tag="mu", name="pmu")
                    psq = ps.tile([128, 512], F32, tag="sqp", name="psq")
                    nc.tensor.matmul(pmu[:], smu[t][:], src[:, sl],
                                     start=True, stop=True)
                    nc.tensor.matmul(psq[:], ssq[t][:], sqc[:],
                                     start=True, stop=True)
                    # t1 = gamma*mu - q  (frees pmu early)
                    t1 = tmp.tile([128, 512], BF16, tag="t1", name="t1")
                    nc.vector.scalar_tensor_tensor(
                        t1[:], pmu[:], gcol[t][:], src[:, sl],
                        op0=Alu.mult, op1=Alu.subtract)
                    sq2 = tmp.tile([128, 512], F32, tag="sq2", name="sq2")
                    nc.scalar.activation(sq2[:], pmu[:], Act.Square)
                    varr = tmp.tile([128, 512], F32, tag="va", name="varr")
                    nc.vector.tensor_sub(varr[:], psq[:], sq2[:])
                    # rs = rsqrt(var + eps) = exp(-0.5 * ln(var + eps))
                    lva = tmp.tile([128, 512], F32, tag="lva", name="lva")
                    nc.scalar.activation(lva[:], varr[:], Act.Ln, bias=epsb[:])
                    rs = tmp.tile([128, 512], BF16, tag="rs", name="rs")
                    nc.scalar.activation(rs[:], lva[:], Act.Exp, scale=-0.5)
                    # q_hat = -t1 * rs  (bf16 out)
                    nc.vector.scalar_tensor_tensor(
                        src[:, sl], t1[:], -1.0, rs[:],
                        op0=Alu.mult, op1=Alu.mult)

                for nch in range(NCH):
                    sl = slice(nch * 512, (nch + 1) * 512)
                    xt = xp.tile([128, CC * 512], BF16, tag="xt", name="xt")
                    # chunk 0 loads via the scalar queue (right behind the
                    # first wqk half) so the PE can start ~2us in; the
                    # second halves of each follow
                    if nch == 0:
                        xt_r3 = xt[:].rearrange("p (cc n) -> p cc n", cc=CC)
                        nc.scalar.dma_start(xt_r3[:, 0:CC // 2],
                                            xT_r[:, 0:CC // 2, sl])
                        ccs2 = slice(CC // 2, CC)
                        nc.scalar.dma_start(
                            wqk[:].rearrange("p (cc o) -> p cc o",
                                             cc=CC)[:, ccs2],
                            wqk_r[:, ccs2])
                        nc.scalar.dma_start(xt_r3[:, ccs2],
                                            xT_r[:, ccs2, sl])
                    else:
                        nc.sync.dma_start(
                            xt[:].rearrange("p (cc n) -> p cc n", cc=CC),
                            xT_r[:, :, sl])
                    # job-major: finish one q/k projection, start its LN
                    # stats chain while the next job's matmuls stream
                    for j, (dest, woff) in enumerate(qk_jobs):
                        acc = ps.tile([128, 512], F32, tag=f"acc{j}",
                                      name=f"acc{j}")
                        for cc in range(CC):
                            nc.tensor.matmul(
                                acc[:],
                                wqk[:, cc * 512 + woff:cc * 512 + woff + 128],
                                xt[:, cc * 512:(cc + 1) * 512],
                                start=(cc == 0), stop=(cc == CC - 1))
                        # copy out of PSUM (cast to bf16); DVE and ACT split
                        if j % 2 == 0:
                            nc.vector.tensor_copy(dest[:, sl], acc[:])
                        else:
                            nc.scalar.copy(dest[:, sl], acc[:])
                        ln_stats(dest, sl, 0 if dest in (qk[0], qk[1]) else 1)
                    vaccs = [ps.tile([128, 512], F32, tag=f"vacc{j}",
                                     name=f"vacc{j}") for j in range(2)]
                    for q in range(4):      # n-tiles within chunk -> v
                        vacc = vaccs[q // 2]
                        # PSUM start=True zeroes the whole bank, so only
                        # the first group in each shared bank sets it;
                        # the second rides on that zeroing.
                        for cc in range(CC):
                            nc.tensor.matmul(
                                vacc[:, (q % 2) * 256:(q % 2) * 256 + 256],
                                xt[:, cc * 512 + q * 128:cc * 512 + q * 128 + 128],
                                wv[:, cc * 256:(cc + 1) * 256],
                                start=(cc == 0 and q % 2 == 0),
                                stop=(cc == CC - 1), skip_group_check=True)
                        if q % 2 == 1:
                            nt0 = nch * 4 + q - 1
                            dst = v_sb[:, nt0 * 320:(nt0 + 2) * 320].rearrange(
                                "p (nt b c) -> p nt b c", b=4, c=80)[:, :, :, 0:64]
                            nc.vector.tensor_copy(
                                dst, vacc[:].rearrange(
                                    "p (nt b d) -> p nt b d", nt=2, b=4))

            # ================= Phase 3: attention =================
            with tc.tile_pool(name="p3", bufs=2) as p3, \
                 tc.tile_pool(name="p4", bufs=1) as p4, \
                 tc.tile_pool(name="ps3", bufs=2, space="PSUM") as ps3, \
                 tc.tile_pool(name="ps3a", bufs=1, space="PSUM") as ps3a:

                def emit_proj(nt):
                    # y tile: contract onorm (both pairs) against wp
                    py = ps3.tile([128, 1024], F32, tag="s", name="py")
                    for oc in range(2):
                        for p in range(PAIRS):
                            nc.tensor.matmul(
                                py[:, oc * 512:(oc + 1) * 512],
                                onorm[p][:, nt * 128:(nt + 1) * 128],
                                wp[:, p * 1024 + oc * 512:p * 1024 + (oc + 1) * 512],
                                start=(p == 0), stop=(p == PAIRS - 1))
                    yt = p4.tile([128, 1024], BF16, tag="yt", bufs=3, name="yt")
                    nc.vector.tensor_copy(yt[:], py[:])
                    # alternate trigger queues so the per-tile y DMAs don't
                    # serialize into a tail (gpsimd is idle by this point)
                    eng = nc.sync if nt % 2 == 0 else nc.gpsimd
                    eng.dma_start(y_d[nt * 128:(nt + 1) * 128, :], yt[:])

                for nh in range(2):            # halves of n (1024 each)
                    for p in range(PAIRS):
                        qt, kt = qk[p], qk[2 + p]
                        nsl = slice(nh * 1024, (nh + 1) * 1024)
                        pending = []
                        poh = [ps3a.tile([128, 1024], F32, tag=f"po{h}",
                                         name=f"po{h}") for h in range(2)]

                        def emit_av(av_mt, h, eS):
                            off = (p * 2 + h) * 80
                            vsl = v_sb[:, av_mt * 320 + off:
                                       av_mt * 320 + off + 65]
                            for nq in range(2):
                                nc.tensor.matmul(
                                    poh[h][0:65, nq * 512:(nq + 1) * 512],
                                    vsl, eS[:, nq * 512:(nq + 1) * 512],
                                    start=(av_mt == 0), stop=(av_mt == NT - 1))

                        # software-pipelined: S(mt) streams while exp(mt-1)
                        # finishes, so AV(mt-1) never blocks the in-order PE
                        # queue on the ACT engine's latency
                        eSq = []
                        for mt in range(NT):
                            for h in range(2):     # head halves (rows 0/64)
                                hs = slice(h * 64, (h + 1) * 64)
                                psS = ps3.tile([128, 1024], F32, tag="s",
                                               name="psS")
                                if KEEPWARM:
                                    # fill the ACT-paced slack so the HAM's
                                    # idle window never fires; S's start=True
                                    # wipes the bank right after
                                    nc.tensor.matmul(
                                        psS[0:64, 0:512], warmsrc[:, 0:64],
                                        warmsrc[:], start=True, stop=True,
                                        skip_group_check=True)
                                for nq in range(2):
                                    nc.tensor.matmul(
                                        psS[:, nq * 512:(nq + 1) * 512],
                                        kt[hs, mt * 128:(mt + 1) * 128],
                                        qt[hs, nh * 1024 + nq * 512:
                                           nh * 1024 + (nq + 1) * 512],
                                        start=True, stop=True)
                                eS = p3.tile([128, 1024], BF16, tag="eS",
                                             bufs=4, name="eS")
                                nc.scalar.activation(eS[:], psS[:], Act.Exp,
                                                     scale=float(SCALE))
                                eSq.append((mt, h, eS))
                                if not PIPELINED_AV:
                                    emit_av(*eSq.pop(0))
                            while len(eSq) > 2:
                                emit_av(*eSq.pop(0))
                            if pending and mt % 2 == 1:
                                emit_proj(pending.pop(0))
                        while eSq:
                            emit_av(*eSq.pop(0))
                        # stage o_unnorm+den out of PSUM immediately (frees
                        # poh for the next iteration), then normalize from
                        # SBUF off the PE's critical path
                        for h in range(2):
                            nc.vector.tensor_copy(ou[h][0:65, :],
                                                  poh[h][0:65, :])
                        if KEEPWARM:
                            # dependency-free fillers bridge the denominator
                            # chain so the HAM's idle window doesn't fire at
                            # the iteration boundary (observed: 60-80us cold
                            # stretches start exactly here)
                            wtile = ps3.tile([128, 1024], F32, tag="s",
                                             name="wfill")
                            for _ in range(6):
                                nc.tensor.matmul(
                                    wtile[0:64, 0:512], warmsrc[:, 0:64],
                                    warmsrc[:], start=True, stop=True,
                                    skip_group_check=True)
                        for h in range(2):
                            pb = ps3.tile([128, 1024], F32, tag="s",
                                          name=f"pb{h}")
                            for nq in range(2):
                                nc.tensor.matmul(
                                    pb[0:64, nq * 512:(nq + 1) * 512], bsel[:],
                                    ou[h][:, nq * 512:(nq + 1) * 512],
                                    start=True, stop=True)
                            ld = p3.tile([128, 1024], F32, tag=f"ld{h}",
                                         name=f"ld{h}")
                            nc.scalar.activation(ld[0:64, :], pb[0:64, :],
                                                 Act.Ln)
                            rd = p3.tile([128, 1024], BF16, tag=f"rd{h}",
                                         name=f"rd{h}")
                            nc.scalar.activation(rd[0:64, :], ld[0:64, :],
                                                 Act.Exp, scale=-1.0)
                            if h == 0:
                                nc.vector.tensor_mul(
                                    onorm[p][0:64, nsl],
                                    ou[0][0:64, :], rd[0:64, :])
                            else:
                                tmpB = p3.tile([128, 1024], BF16, tag="tmpB",
                                               name="tmpB")
                                nc.vector.tensor_mul(
                                    tmpB[0:64, :], ou[1][0:64, :], rd[0:64, :])
                                nc.sync.dma_start(
                                    onorm[p][64:128, nsl], tmpB[0:64, :])
                    for nt in range(nh * 8, (nh + 1) * 8):
                        emit_proj(nt)

    nc.compile()
    return nc


def _prep_core_inputs(x, W_qkv, q_gamma, k_gamma, W_proj):
    """Host-side sharding + layout prep. Returns list of 8 in_maps."""
    f32 = np.float32
    bf16 = BF16NP
    blkdiag = np.kron(np.eye(2, dtype=f32), np.ones((64, 64), f32))
    bsel = np.zeros((128, 64), f32)
    bsel[64, :] = 1.0
    bsel = bsel.astype(bf16)
    in_maps = []
    for core in range(N_CORES):
        b, g = core // 4, core % 4
        heads = [4 * g + j for j in range(HEADS_PER_CORE)]
        qcols = np.concatenate(
            [(W_qkv[h * HD:(h + 1) * HD, :] * q_gamma[:, None]).T for h in heads],
            axis=1)
        kcols = np.concatenate(
            [(W_qkv[DIM + h * HD:DIM + (h + 1) * HD, :] * k_gamma[:, None]).T
             for h in heads], axis=1)
        wqk = np.ascontiguousarray(
            np.concatenate([qcols, kcols], axis=1), dtype=f32).astype(bf16)
        wv = np.ascontiguousarray(
            np.concatenate(
                [W_qkv[2 * DIM + h * HD:2 * DIM + (h + 1) * HD, :].T
                 for h in heads], axis=1), dtype=f32).astype(bf16)
        wp = np.ascontiguousarray(
            W_proj[:, heads[0] * HD:(heads[-1] + 1) * HD].T,
            dtype=f32).astype(bf16)
        g2q = np.tile(q_gamma, 2).astype(f32)
        g2k = np.tile(k_gamma, 2).astype(f32)
        in_maps.append({
            "xT": np.ascontiguousarray(x[b].T, dtype=f32).astype(bf16),
            "wqk": wqk, "wv": wv, "wp": wp,
            "smu_q": (blkdiag * (1.0 / (64.0 * g2q))[:, None]).astype(bf16),
            "ssq_q": (blkdiag * (1.0 / (64.0 * g2q * g2q))[:, None]).astype(bf16),
            "smu_k": (blkdiag * (1.0 / (64.0 * g2k))[:, None]).astype(bf16),
            "ssq_k": (blkdiag * (1.0 / (64.0 * g2k * g2k))[:, None]).astype(bf16),
            "bsel": bsel,
            "gq": g2q[:, None].copy(), "gk": g2k[:, None].copy(),
            "ones": np.ones((128, 64), bf16),
        })
    return in_maps


def _numpy_fallback(x, W_qkv, q_gamma, q_beta, k_gamma, k_beta, W_proj, b_proj):
    def ln(t, gamma, beta):
        mu = t.mean(-1, keepdims=True)
        var = ((t - mu) ** 2).mean(-1, keepdims=True)
        return (t - mu) / np.sqrt(var + EPS) * gamma + beta
    Bs, Ns, C = x.shape
    qkv = np.einsum('bnc,oc->bno', x, W_qkv)
    qkv = qkv.reshape(Bs, Ns, 3, HEADS, HD).transpose(2, 0, 3, 1, 4)
    q, k, v = ln(qkv[0], q_gamma, q_beta), ln(qkv[1], k_gamma, k_beta), qkv[2]
    s = np.einsum('bhnd,bhmd->bhnm', q * SCALE, k)
    s = np.exp(s - s.max(-1, keepdims=True))
    p = s / s.sum(-1, keepdims=True)
    o = np.einsum('bhnm,bhmd->bhnd', p, v)
    o = o.transpose(0, 2, 1, 3).reshape(Bs, Ns, C)
    return (np.einsum('bnc,oc->bno', o, W_proj) + b_proj).astype(np.float32)


def kernel(x, W_qkv, q_gamma, q_beta, k_gamma, k_beta, W_proj, b_proj):
    x = np.asarray(x, np.float32)
    W_qkv = np.asarray(W_qkv, np.float32)
    q_gamma = np.asarray(q_gamma, np.float32)
    q_beta = np.asarray(q_beta, np.float32)
    k_gamma = np.asarray(k_gamma, np.float32)
    k_beta = np.asarray(k_beta, np.float32)
    W_proj = np.asarray(W_proj, np.float32)
    b_proj = np.asarray(b_proj, np.float32)

    if np.any(q_beta != 0) or np.any(k_beta != 0):
        # beta terms are not wired into the device kernel (reference always
        # uses beta = 0); fall back to exact host computation
        return _numpy_fallback(x, W_qkv, q_gamma, q_beta, k_gamma, k_beta,
                               W_proj, b_proj)

    from concourse import bass_utils

    if "prog" not in _prog_cache:
        _prog_cache["prog"] = _build_program()
    nc = _prog_cache["prog"]

    in_maps = _prep_core_inputs(x, W_qkv, q_gamma, k_gamma, W_proj)
    res = bass_utils.run_bass_kernel_spmd(nc, in_maps, list(range(N_CORES)))

    out = np.empty((B, N, DIM), np.float32)
    for b in range(B):
        acc = res.results[4 * b + 0]["y"].astype(np.float32).copy()
        for g in range(1, 4):
            acc += res.results[4 * b + g]["y"]
        out[b] = acc + b_proj
    return out


# revision 81
# speedup vs baseline: 1.0738x; 1.0738x over previous
"""Trainium2 Bass kernel for qk-layernorm attention (dense transformer block).

Sharding: 8 cores = 2 batches x 4 head-groups (4 heads each).  Each core
computes qkv projection (its heads only), qk-layernorm, attention, and a
partial output projection for its head slice; the host sums the 4 partials
per batch and adds b_proj.

v2 (bf16 datapath, ~355-385us vs 533-658us fp32r baseline):
 - all matmul operands bf16 (host pre-casts x/W); PSUM accumulation fp32.
   bf16 halves LDWEIGHTS time and DMA bytes; the PE HAM clock gate
   (K=4/8 cold / 8/8 warm) dominates either way, so stream-cycle count
   and PE-queue continuity are what matter.
 - v projected directly into [n, d]-per-head layout (no PE transposes)
 - single ACT table (natural_log_exp_and_others): exp for softmax,
   rsqrt(v)=exp(-0.5*ln(v+eps)) for LN, 1/den=exp(-ln(den)) for softmax
   denominators -- no DVE reciprocal (was 105us), no table thrash
   (was 49 ACT_TABLE_LOADs = 63us)
 - phase 1 job-major: each q/k projection's LN-stats chain drains while
   the next job's matmuls stream; PSUM copies split DVE/ACT
 - attention inner loop software-pipelined: S(mt) streams while exp(mt-1)
   finishes so attn@v never head-of-line-blocks the in-order PE queue;
   o_unnorm+den staged to SBUF immediately to free the accumulator banks
 - startup DMAs spread across scalar/sync/gpsimd queues (lead-in 30->15us)
 - fp8e4m3 DoubleRow attn@v was tried (−50us) but the summed per-head-
   group errors land at 2.5e-2 > the 2e-2 gate; bf16 keeps 5.5e-3.
"""

import numpy as np
import ml_dtypes

DIM = 1024
HEADS = 16
HD = 64
B = 2
N = 2048
EPS = 1e-6
N_CORES = 8
HEADS_PER_CORE = 4
PAIRS = 2          # head pairs per core
CC = 8             # contraction chunks of 128 over DIM
NT = N // 128      # 16 n/m tiles
NCH = N // 512     # 4 chunks of 512
SCALE = HD ** -0.5

BF16NP = ml_dtypes.bfloat16
PIPELINED_AV = True   # lag attn@v one m-tile behind S to hide exp latency
WARMUP_MM = 55        # dummy matmuls spanning the DMA lead-in (HAM pre-warm)
KEEPWARM = True       # dummy matmul per attn step to hold the HAM at K=8/8   # lag attn@v one m-tile behind S to hide exp latency

_prog_cache = {}


def _pin_act_table():
    """Force all ACT activations onto the natural_log_exp_and_others table.

    The greedy table-selection pass otherwise thrashes between
    exp_and_others (Exp) and natural_log (Ln) -- 49 ACT_TABLE_LOADs x
    1.28us in the traced kernel.  We keep table order/ids intact (ids are
    positional into act_info.json) but report every other table as empty
    so the one table containing {Exp, Ln, Square} serves everything.
    """
    import concourse.bacc as bacc_mod
    if getattr(bacc_mod.get_activation_tables, "_pinned", False):
        return
    orig = bacc_mod.get_activation_tables
    pref = "natural_log_exp_and_others"

    def pinned(arch):
        t = orig(arch)
        if pref not in t:
            return t
        return {name: (funcs if name == pref else set())
                for name, funcs in t.items()}

    pinned._pinned = True
    bacc_mod.get_activation_tables = pinned


def _build_program():
    import concourse.bass as bass
    import concourse.tile as tile
    from concourse import mybir, bacc

    _pin_act_table()

    F32 = mybir.dt.float32
    BF16 = mybir.dt.bfloat16
    Act = mybir.ActivationFunctionType
    Alu = mybir.AluOpType

    nc = bacc.Bacc("TRN2", target_bir_lowering=False, debug=False,
                   num_devices=N_CORES)

    # ---- DRAM I/O ----
    xT_d = nc.dram_tensor("xT", [DIM, N], BF16, kind="ExternalInput").ap()
    wqk_d = nc.dram_tensor("wqk", [DIM, 512], BF16, kind="ExternalInput").ap()
    wv_d = nc.dram_tensor("wv", [DIM, 256], BF16, kind="ExternalInput").ap()
    wp_d = nc.dram_tensor("wp", [256, DIM], BF16, kind="ExternalInput").ap()
    smu_q_d = nc.dram_tensor("smu_q", [128, 128], BF16, kind="ExternalInput").ap()
    ssq_q_d = nc.dram_tensor("ssq_q", [128, 128], BF16, kind="ExternalInput").ap()
    smu_k_d = nc.dram_tensor("smu_k", [128, 128], BF16, kind="ExternalInput").ap()
    ssq_k_d = nc.dram_tensor("ssq_k", [128, 128], BF16, kind="ExternalInput").ap()
    bsel_d = nc.dram_tensor("bsel", [128, 64], BF16, kind="ExternalInput").ap()
    gq_d = nc.dram_tensor("gq", [128, 1], F32, kind="ExternalInput").ap()
    gk_d = nc.dram_tensor("gk", [128, 1], F32, kind="ExternalInput").ap()
    ones_d = nc.dram_tensor("ones", [128, 64], BF16, kind="ExternalInput").ap()
    # y partials leave the core in bf16 (halves the 8MB output DMA); the
    # host sums the 4 head-group partials in fp32
    y_d = nc.dram_tensor("y", [N, DIM], BF16, kind="ExternalOutput").ap()

    with tile.TileContext(nc) as tc:
        with tc.tile_pool(name="wts", bufs=1) as wts, \
             tc.tile_pool(name="persist", bufs=1) as persist:
            # ---- persistent SBUF tensors ----
            # Startup DMAs are spread across engine queues so issue
            # overhead (~1us per dma_start) doesn't serialize the lead-in:
            # scalar: wqk (gates first matmuls); vector: xt chunk 0;
            # sync: wv + the rest; gpsimd: memsets.
            # zero source for HAM warm-up matmuls (memset lands ~0.5us in,
            # long before the first weight DMA completes)
            warmsrc = wts.tile([128, 512], BF16)
            nc.gpsimd.memset(warmsrc[:], 0.0)
            wqk = wts.tile([128, CC * 512], BF16)     # [c-chunk, 4 o-tiles x 128]
            wqk_r = wqk_d.rearrange("(cc p) o -> p cc o", p=128)
            ccs = slice(0, CC // 2)
            nc.scalar.dma_start(
                wqk[:].rearrange("p (cc o) -> p cc o", cc=CC)[:, ccs],
                wqk_r[:, ccs])
            wv = wts.tile([128, CC * 256], BF16)
            nc.sync.dma_start(wv[:].rearrange("p (cc o) -> p cc o", cc=CC),
                              wv_d.rearrange("(cc p) o -> p cc o", p=128))
            wp = wts.tile([128, 2 * DIM], BF16)
            smu = [wts.tile([128, 128], BF16, tag=f"smu{t}", name=f"smu{t}") for t in range(2)]
            ssq = [wts.tile([128, 128], BF16, tag=f"ssq{t}", name=f"ssq{t}") for t in range(2)]
            nc.sync.dma_start(smu[0][:], smu_q_d[:])
            nc.sync.dma_start(ssq[0][:], ssq_q_d[:])
            nc.sync.dma_start(smu[1][:], smu_k_d[:])
            nc.sync.dma_start(ssq[1][:], ssq_k_d[:])
            gcol = [wts.tile([128, 1], F32, tag=f"g{t}", name=f"g{t}") for t in range(2)]
            nc.sync.dma_start(gcol[0][:], gq_d[:])
            nc.sync.dma_start(gcol[1][:], gk_d[:])
            ones = wts.tile([128, 64], BF16)
            nc.sync.dma_start(ones[:], ones_d[:])
            bsel = wts.tile([128, 64], BF16)
            nc.sync.dma_start(bsel[:], bsel_d[:])
            # o_unnorm staging (rows 65-127 stay zero for the bsel matmul)
            ou = [wts.tile([128, 1024], BF16, tag=f"ou{h}", name=f"ou{h}")
                  for h in range(2)]
            nc.gpsimd.memset(ou[0][:], 0.0)
            nc.gpsimd.memset(ou[1][:], 0.0)
            epsb = wts.tile([128, 1], F32)
            nc.gpsimd.memset(epsb[:], EPS)

            # qk[0],qk[1]: q pair tiles; qk[2],qk[3]: k pair tiles  [d-pair, n]
            qk = [persist.tile([128, N], BF16, tag=f"qk{i}", name=f"qk{i}") for i in range(4)]
            # v in [m, per-nt: 4 heads x (64 v | 1 ones | 15 pad)] layout
            v_sb = persist.tile([128, NT * 320], BF16)
            nc.gpsimd.memset(v_sb[:], 0.0)
            onorm = [persist.tile([128, N], BF16, tag=f"on{p}", name=f"on{p}") for p in range(PAIRS)]
            v_ones = v_sb[:].rearrange("p (nt b c) -> p nt b c", b=4, c=80)[:, :, :, 64:65]
            nc.gpsimd.dma_start(
                v_ones, ones_d.rearrange("p (nt b) -> p nt b", b=4)[:, :, :, None])
            # wp is only needed by the output projection (late)
            nc.gpsimd.dma_start(wp[:].rearrange("p (pc o) -> p pc o", pc=2),
                                wp_d.rearrange("(pc p) o -> p pc o", p=128))

            # ================= Phase 1: qkv projection + qk layernorm =====
            xT_r = xT_d.rearrange("(cc p) n -> p cc n", p=128)
            # jobs: (dest qk tile, col offset in wqk)
            qk_jobs = [(qk[0], 0), (qk[2], 256), (qk[1], 128), (qk[3], 384)]

            with tc.tile_pool(name="xT", bufs=2) as xp, \
                 tc.tile_pool(name="p1tmp", bufs=2) as tmp, \
                 tc.tile_pool(name="ps_p1", bufs=1, space="PSUM") as ps:
                # The PE HAM clock gate starts at K=4/8 (1.2 GHz) and only
                # reaches 8/8 after ~3.4us of sustained activity.  Burn the
                # DMA lead-in on dummy matmuls so phase 1 starts warm.
                if WARMUP_MM:
                    warm = ps.tile([128, 512], F32, tag="acc0", name="warm")
                    for _ in range(WARMUP_MM):
                        nc.tensor.matmul(warm[0:64, :], warmsrc[:, 0:64],
                                         warmsrc[:], start=True, stop=True,
                                         skip_group_check=True)

                def ln_stats(src, sl, t):
                    # qk layernorm for one 512-token chunk of one q/k pair
                    sqc = tmp.tile([128, 512], BF16, tag="sqc", name="sqc")
                    nc.vector.tensor_mul(sqc[:], src[:, sl], src[:, sl])
                    pmu = ps.tile([128, 512], F32, tag="mu", name="pmu")
                    psq = ps.tile([128, 512], F32, tag="sqp", name="psq")
                    nc.tensor.matmul(pmu[:], smu[t][:], src[:, sl],
                                     start=True, stop=True)
                    nc.tensor.matmul(psq[:], ssq[t][:], sqc[:],
                                     start=True, stop=True)
                    # t1 = gamma*mu - q  (frees pmu early)
                    t1 = tmp.tile([128, 512], F32, tag="t1", name="t1")
                    nc.vector.scalar_tensor_tensor(
                        t1[:], pmu[:], gcol[t][:], src[:, sl],
                        op0=Alu.mult, op1=Alu.subtract)
                    sq2 = tmp.tile([128, 512], F32, tag="sq2", name="sq2")
                    nc.scalar.activation(sq2[:], pmu[:], Act.Square)
                    varr = tmp.tile([128, 512], F32, tag="va", name="varr")
                    nc.vector.tensor_sub(varr[:], psq[:], sq2[:])
                    # rs = rsqrt(var + eps) = exp(-0.5 * ln(var + eps))
                    lva = tmp.tile([128, 512], F32, tag="lva", name="lva")
                    nc.scalar.activation(lva[:], varr[:], Act.Ln, bias=epsb[:])
                    rs = tmp.tile([128, 512], F32, tag="rs", name="rs")
                    nc.scalar.activation(rs[:], lva[:], Act.Exp, scale=-0.5)
                    # q_hat = -t1 * rs  (bf16 out)
                    nc.vector.scalar_tensor_tensor(
                        src[:, sl], t1[:], -1.0, rs[:],
                        op0=Alu.mult, op1=Alu.mult)

                for nch in range(NCH):
                    sl = slice(nch * 512, (nch + 1) * 512)
                    xt = xp.tile([128, CC * 512], BF16, tag="xt", name="xt")
                    # chunk 0 loads via the scalar queue (right behind the
                    # first wqk half) so the PE can start ~2us in; the
                    # second halves of each follow
                    if nch == 0:
                        xt_r3 = xt[:].rearrange("p (cc n) -> p cc n", cc=CC)
                        nc.scalar.dma_start(xt_r3[:, 0:CC // 2],
                                            xT_r[:, 0:CC // 2, sl])
                        ccs2 = slice(CC // 2, CC)
                        nc.scalar.dma_start(
                            wqk[:].rearrange("p (cc o) -> p cc o",
                                             cc=CC)[:, ccs2],
                            wqk_r[:, ccs2])
                        nc.scalar.dma_start(xt_r3[:, ccs2],
                                            xT_r[:, ccs2, sl])
                    else:
                        nc.sync.dma_start(
                            xt[:].rearrange("p (cc n) -> p cc n", cc=CC),
                            xT_r[:, :, sl])
                    # job-major: finish one q/k projection, start its LN
                    # stats chain while the next job's matmuls stream
                    for j, (dest, woff) in enumerate(qk_jobs):
                        acc = ps.tile([128, 512], F32, tag=f"acc{j}",
                                      name=f"acc{j}")
                        for cc in range(CC):
                            nc.tensor.matmul(
                                acc[:],
                                wqk[:, cc * 512 + woff:cc * 512 + woff + 128],
                                xt[:, cc * 512:(cc + 1) * 512],
                                start=(cc == 0), stop=(cc == CC - 1))
                        # copy out of PSUM (cast to bf16); DVE and ACT split
                        if j % 2 == 0:
                            nc.vector.tensor_copy(dest[:, sl], acc[:])
                        else:
                            nc.scalar.copy(dest[:, sl], acc[:])
                        ln_stats(dest, sl, 0 if dest in (qk[0], qk[1]) else 1)
                    vaccs = [ps.tile([128, 512], F32, tag=f"vacc{j}",
                                     name=f"vacc{j}") for j in range(2)]
                    for q in range(4):      # n-tiles within chunk -> v
                        vacc = vaccs[q // 2]
                        # PSUM start=True zeroes the whole bank, so only
                        # the first group in each shared bank sets it;
                        # the second rides on that zeroing.
                        for cc in range(CC):
                            nc.tensor.matmul(
                                vacc[:, (q % 2) * 256:(q % 2) * 256 + 256],
                                xt[:, cc * 512 + q * 128:cc * 512 + q * 128 + 128],
                                wv[:, cc * 256:(cc + 1) * 256],
                                start=(cc == 0 and q % 2 == 0),
                                stop=(cc == CC - 1), skip_group_check=True)
                        if q % 2 == 1:
                            nt0 = nch * 4 + q - 1
                            dst = v_sb[:, nt0 * 320:(nt0 + 2) * 320].rearrange(
                                "p (nt b c) -> p nt b c", b=4, c=80)[:, :, :, 0:64]
                            nc.vector.tensor_copy(
                                dst, vacc[:].rearrange(
                                    "p (nt b d) -> p nt b d", nt=2, b=4))

            # ================= Phase 3: attention =================
            with tc.tile_pool(name="p3", bufs=2) as p3, \
                 tc.tile_pool(name="p4", bufs=1) as p4, \
                 tc.tile_pool(name="ps3", bufs=2, space="PSUM") as ps3, \
                 tc.tile_pool(name="ps3a", bufs=1, space="PSUM") as ps3a:

                def emit_proj(nt):
                    # y tile: contract onorm (both pairs) against wp
                    py = ps3.tile([128, 1024], F32, tag="s", name="py")
                    for oc in range(2):
                        for p in range(PAIRS):
                            nc.tensor.matmul(
                                py[:, oc * 512:(oc + 1) * 512],
                                onorm[p][:, nt * 128:(nt + 1) * 128],
                                wp[:, p * 1024 + oc * 512:p * 1024 + (oc + 1) * 512],
                                start=(p == 0), stop=(p == PAIRS - 1))
                    yt = p4.tile([128, 1024], BF16, tag="yt", bufs=3, name="yt")
                    nc.vector.tensor_copy(yt[:], py[:])
                    # alternate trigger queues so the per-tile y DMAs don't
                    # serialize into a tail (gpsimd is idle by this point)
                    eng = nc.sync if nt % 2 == 0 else nc.gpsimd
                    eng.dma_start(y_d[nt * 128:(nt + 1) * 128, :], yt[:])

                for nh in range(2):            # halves of n (1024 each)
                    for p in range(PAIRS):
                        qt, kt = qk[p], qk[2 + p]
                        nsl = slice(nh * 1024, (nh + 1) * 1024)
                        pending = []
                        poh = [ps3a.tile([128, 1024], F32, tag=f"po{h}",
                                         name=f"po{h}") for h in range(2)]

                        def emit_av(av_mt, h, eS):
                            off = (p * 2 + h) * 80
                            vsl = v_sb[:, av_mt * 320 + off:
                                       av_mt * 320 + off + 65]
                            for nq in range(2):
                                nc.tensor.matmul(
                                    poh[h][0:65, nq * 512:(nq + 1) * 512],
                                    vsl, eS[:, nq * 512:(nq + 1) * 512],
                                    start=(av_mt == 0), stop=(av_mt == NT - 1))

                        # software-pipelined: S(mt) streams while exp(mt-1)
                        # finishes, so AV(mt-1) never blocks the in-order PE
                        # queue on the ACT engine's latency
                        eSq = []
                        for mt in range(NT):
                            for h in range(2):     # head halves (rows 0/64)
                                hs = slice(h * 64, (h + 1) * 64)
                                psS = ps3.tile([128, 1024], F32, tag="s",
                                               name="psS")
                                if KEEPWARM:
                                    # fill the ACT-paced slack so the HAM's
                                    # idle window never fires; S's start=True
                                    # wipes the bank right after
                                    nc.tensor.matmul(
                                        psS[0:64, 0:512], warmsrc[:, 0:64],
                                        warmsrc[:], start=True, stop=True,
                                        skip_group_check=True)
                                for nq in range(2):
                                    nc.tensor.matmul(
                                        psS[:, nq * 512:(nq + 1) * 512],
                                        kt[hs, mt * 128:(mt + 1) * 128],
                                        qt[hs, nh * 1024 + nq * 512:
                                           nh * 1024 + (nq + 1) * 512],
                                        start=True, stop=True)
                                eS = p3.tile([128, 1024], BF16, tag="eS",
                                             bufs=4, name="eS")
                                nc.scalar.activation(eS[:], psS[:], Act.Exp,
                                                     scale=float(SCALE))
                                eSq.append((mt, h, eS))
                                if not PIPELINED_AV:
                                    emit_av(*eSq.pop(0))
                            while len(eSq) > 2:
                                emit_av(*eSq.pop(0))
                            if pending and mt % 2 == 1:
                                emit_proj(pending.pop(0))
                        while eSq:
                            emit_av(*eSq.pop(0))
                        # stage o_unnorm+den out of PSUM immediately (frees
                        # poh for the next iteration), then normalize from
                        # SBUF off the PE's critical path
                        for h in range(2):
                            nc.vector.tensor_copy(ou[h][0:65, :],
                                                  poh[h][0:65, :])
                        if KEEPWARM:
                            # dependency-free fillers bridge the denominator
                            # chain so the HAM's idle window doesn't fire at
                            # the iteration boundary (observed: 60-80us cold
                            # stretches start exactly here)
                            wtile = ps3.tile([128, 1024], F32, tag="s",
                                             name="wfill")
                            for _ in range(6):
                                nc.tensor.matmul(
                                    wtile[0:64, 0:512], warmsrc[:, 0:64],
                                    warmsrc[:], start=True, stop=True,
                                    skip_group_check=True)
                        for h in range(2):
                            pb = ps3.tile([128, 1024], F32, tag="s",
                                          name=f"pb{h}")
                            for nq in range(2):
                                nc.tensor.matmul(
                                    pb[0:64, nq * 512:(nq + 1) * 512], bsel[:],
                                    ou[h][:, nq * 512:(nq + 1) * 512],
                                    start=True, stop=True)
                            ld = p3.tile([128, 1024], F32, tag=f"ld{h}",
                                         name=f"ld{h}")
                            nc.scalar.activation(ld[0:64, :], pb[0:64, :],
                                                 Act.Ln)
                            rd = p3.tile([128, 1024], F32, tag=f"rd{h}",
                                         name=f"rd{h}")
                            nc.scalar.activation(rd[0:64, :], ld[0:64, :],
                                                 Act.Exp, scale=-1.0)
                            if h == 0:
                                nc.vector.tensor_mul(
                                    onorm[p][0:64, nsl],
                                    ou[0][0:64, :], rd[0:64, :])
                            else:
                                tmpB = p3.tile([128, 1024], BF16, tag="tmpB",
                                               name="tmpB")
                                nc.vector.tensor_mul(
                                    tmpB[0:64, :], ou[1][0:64, :], rd[0:64, :])
                                nc.sync.dma_start(
                                    onorm[p][64:128, nsl], tmpB[0:64, :])
                    for nt in range(nh * 8, (nh + 1) * 8):
                        emit_proj(nt)

    nc.compile()
    return nc


def _prep_core_inputs(x, W_qkv, q_gamma, k_gamma, W_proj):
    """Host-side sharding + layout prep. Returns list of 8 in_maps."""
    f32 = np.float32
    bf16 = BF16NP
    blkdiag = np.kron(np.eye(2, dtype=f32), np.ones((64, 64), f32))
    bsel = np.zeros((128, 64), f32)
    bsel[64, :] = 1.0
    bsel = bsel.astype(bf16)
    in_maps = []
    for core in range(N_CORES):
        b, g = core // 4, core % 4
        heads = [4 * g + j for j in range(HEADS_PER_CORE)]
        qcols = np.concatenate(
            [(W_qkv[h * HD:(h + 1) * HD, :] * q_gamma[:, None]).T for h in heads],
            axis=1)
        kcols = np.concatenate(
            [(W_qkv[DIM + h * HD:DIM + (h + 1) * HD, :] * k_gamma[:, None]).T
             for h in heads], axis=1)
        wqk = np.ascontiguousarray(
            np.concatenate([qcols, kcols], axis=1), dtype=f32).astype(bf16)
        wv = np.ascontiguousarray(
            np.concatenate(
                [W_qkv[2 * DIM + h * HD:2 * DIM + (h + 1) * HD, :].T
                 for h in heads], axis=1), dtype=f32).astype(bf16)
        wp = np.ascontiguousarray(
            W_proj[:, heads[0] * HD:(heads[-1] + 1) * HD].T,
            dtype=f32).astype(bf16)
        g2q = np.tile(q_gamma, 2).astype(f32)
        g2k = np.tile(k_gamma, 2).astype(f32)
        in_maps.append({
            "xT": np.ascontiguousarray(x[b].T, dtype=f32).astype(bf16),
            "wqk": wqk, "wv": wv, "wp": wp,
            "smu_q": (blkdiag * (1.0 / (64.0 * g2q))[:, None]).astype(bf16),
            "ssq_q": (blkdiag * (1.0 / (64.0 * g2q * g2q))[:, None]).astype(bf16),
            "smu_k": (blkdiag * (1.0 / (64.0 * g2k))[:, None]).astype(bf16),
            "ssq_k": (blkdiag * (1.0 / (64.0 * g2k * g2k))[:, None]).astype(bf16),
            "bsel": bsel,
            "gq": g2q[:, None].copy(), "gk": g2k[:, None].copy(),
            "ones": np.ones((128, 64), bf16),
        })
    return in_maps


def _numpy_fallback(x, W_qkv, q_gamma, q_beta, k_gamma, k_beta, W_proj, b_proj):
    def ln(t, gamma, beta):
        mu = t.mean(-1, keepdims=True)
        var = ((t - mu) ** 2).mean(-1, keepdims=True)
        return (t - mu) / np.sqrt(var + EPS) * gamma + beta
    Bs, Ns, C = x.shape
    qkv = np.einsum('bnc,oc->bno', x, W_qkv)
    qkv = qkv.reshape(Bs, Ns, 3, HEADS, HD).transpose(2, 0, 3, 1, 4)
    q, k, v = ln(qkv[0], q_gamma, q_beta), ln(qkv[1], k_gamma, k_beta), qkv[2]
    s = np.einsum('bhnd,bhmd->bhnm', q * SCALE, k)
    s = np.exp(s - s.max(-1, keepdims=True))
    p = s / s.sum(-1, keepdims=True)
    o = np.einsum('bhnm,bhmd->bhnd', p, v)
    o = o.transpose(0, 2, 1, 3).reshape(Bs, Ns, C)
    return (np.einsum('bnc,oc->bno', o, W_proj) + b_proj).astype(np.float32)


def kernel(x, W_qkv, q_gamma, q_beta, k_gamma, k_beta, W_proj, b_proj):
    x = np.asarray(x, np.float32)
    W_qkv = np.asarray(W_qkv, np.float32)
    q_gamma = np.asarray(q_gamma, np.float32)
    q_beta = np.asarray(q_beta, np.float32)
    k_gamma = np.asarray(k_gamma, np.float32)
    k_beta = np.asarray(k_beta, np.float32)
    W_proj = np.asarray(W_proj, np.float32)
    b_proj = np.asarray(b_proj, np.float32)

    if np.any(q_beta != 0) or np.any(k_beta != 0):
        # beta terms are not wired into the device kernel (reference always
        # uses beta = 0); fall back to exact host computation
        return _numpy_fallback(x, W_qkv, q_gamma, q_beta, k_gamma, k_beta,
                               W_proj, b_proj)

    from concourse import bass_utils

    if "prog" not in _prog_cache:
        _prog_cache["prog"] = _build_program()
    nc = _prog_cache["prog"]

    in_maps = _prep_core_inputs(x, W_qkv, q_gamma, k_gamma, W_proj)
    res = bass_utils.run_bass_kernel_spmd(nc, in_maps, list(range(N_CORES)))

    out = np.empty((B, N, DIM), np.float32)
    for b in range(B):
        acc = res.results[4 * b + 0]["y"].astype(np.float32).copy()
        for g in range(1, 4):
            acc += res.results[4 * b + g]["y"]
        out[b] = acc + b_proj
    return out
